# revision 23
# baseline (speedup 1.0000x reference)
"""Cross-attention kernel for TRN2, SPMD over 8 NeuronCores.

Problem: B=8, SQ=4096, SKV=77, D_EMBED=1024, D_CROSS=768, H=16, DH=64.
  q = x @ wq + bq ; k = y @ wk + bk ; v = y @ wv + bv
  out = softmax(q k^T / 8) v @ wo + bo

Sharding: pure data-parallel over batch (1 batch element per core, no
collectives). Host pre-packs every DRAM operand into the exact
[partition, tile...] layout the device consumes (multi-KB contiguous DMA
lines), and the O-projection emits the output TRANSPOSED (outT[d, q]) so
its PSUM drain is an ACT activation with per-partition bias; the host
transposes back.

Compute dtype: bf16 operands (host-cast), fp32 PSUM accumulation, fp32 out.

Schedule: two-stage software pipeline over 512-wide query chunks.
Iteration i runs pass A for chunk i (Q-projection with paired score
matmuls + exp trailing one granule behind) then pass B for chunk i-1
(PV pairs + normalization muls, O-projection with chunk i's exp-sum
pairs interleaved between O-groups so their reciprocals have a whole
O-phase of DVE slack).

Softmax without max-subtraction (scores are O(5) here; exp stays inside
fp32/bf16 range):
  scoresT[s,q] = k'_h @ q_h^T with k' = (k + bk)/8 folded at k-projection
  e = exp(scoresT)  (bf16)
  rb[d,q] = 1 / (ones^T @ e)  per-head exp-sums broadcast over the head's
            64-row block by an all-ones [77,64] stationary, col-tiled so
            a head PAIR costs one 512-column stream
  aoT[d,q] = (v_h^T @ e) * rb   (normalization commutes with PV)
  outT[n,q] = wo_slice^T @ aoT  (+bo via ACT bias at the PSUM drain)

Engine budget per iteration (steady state): PE ~32.5us (the bottleneck:
projections 27.3, scores/sums/PV 1.7 each), ACT ~16us (exps + qT/outT
drains), DVE ~10us (reciprocals + aoT muls), PSUM 8 banks = pq(4) +
scores(1x2) + sums/PV shared(2x1).
"""

import numpy as np
import ml_dtypes

import concourse.bass as bass
import concourse.mybir as mybir
import concourse.tile as tile
from concourse import bacc
from concourse import bass_utils

F32 = mybir.dt.float32
BF16 = mybir.dt.bfloat16
AF = mybir.ActivationFunctionType

B = 8
SQ = 4096
SKV = 77
D = 1024
DC = 768
H = 16
DH = 64
KT = D // 128    # 8 embed k-tiles
KC = DC // 128   # 6 cross k-tiles
CT = D // 128    # 8 column tiles of the 1024-wide projections
CH = 512         # query chunk
NCH = SQ // CH   # 8 chunks
HP = H // 2      # 8 head pairs

_CACHED = {}


def _build():
    nc = bacc.Bacc("TRN2", target_bir_lowering=False, debug=False, num_devices=B)

    xp = nc.dram_tensor("xp", (128, NCH * KT * CH), BF16, kind="ExternalInput")
    yp = nc.dram_tensor("yp", (128, KC * SKV), BF16, kind="ExternalInput")
    wq_d = nc.dram_tensor("wq", (128, CT * KT * 128), BF16, kind="ExternalInput")
    wk_d = nc.dram_tensor("wk", (128, CT * KC * 128), BF16, kind="ExternalInput")
    wv_d = nc.dram_tensor("wv", (128, KC * D), BF16, kind="ExternalInput")
    wo_d = nc.dram_tensor("wo", (128, KT * D), BF16, kind="ExternalInput")
    bq32_d = nc.dram_tensor("bq32", (128, CT), F32, kind="ExternalInput")
    bk8_d = nc.dram_tensor("bk8", (128, CT), F32, kind="ExternalInput")
    bv_d = nc.dram_tensor("bv", (1, D), BF16, kind="ExternalInput")
    boT_d = nc.dram_tensor("boT", (128, CT), F32, kind="ExternalInput")
    out_d = nc.dram_tensor("out", (D, SQ), F32, kind="ExternalOutput")

    with tile.TileContext(nc) as tc:
        with (
            tc.tile_pool(name="consts", bufs=1) as consts,
            tc.tile_pool(name="wpool", bufs=1) as wpool,
            tc.tile_pool(name="xpool", bufs=2) as xpool,
            tc.tile_pool(name="qpool", bufs=2) as qpool,
            tc.tile_pool(name="epool", bufs=2) as epool,
            tc.tile_pool(name="rbpool", bufs=16) as rbpool,
            tc.tile_pool(name="aopool", bufs=2) as aopool,
            tc.tile_pool(name="opool", bufs=3) as opool,
            tc.tile_pool(name="pq", bufs=3, space="PSUM") as pq,
            tc.tile_pool(name="psc", bufs=1, space="PSUM") as psc,
            tc.tile_pool(name="att", bufs=3, space="PSUM") as att,
        ):
            xr = xp.ap().rearrange("p (c kt q) -> p c kt q", c=NCH, kt=KT)
            wqr = wq_d.ap().rearrange("p (g kt n) -> p g kt n", g=CT, kt=KT)
            wkr = wk_d.ap().rearrange("p (g kt n) -> p g kt n", g=CT, kt=KC)

            # device-generated constants (no DMA dependency)
            ones77r = consts.tile([1, SKV], BF16, tag="ones77r")
            nc.vector.memset(ones77r[:], 1.0)
            ones64 = consts.tile([SKV, 64], BF16, tag="ones64")
            nc.vector.memset(ones64[:], 1.0)
            scr = consts.tile([SKV, CH], BF16, tag="scr")
            nc.vector.memset(scr[:], 0.0)

            # ---- startup DMAs: Qproj(0) inputs first (the single sync HW
            # queue transfers in issue order at ~320 GB/s, so order = priority)
            xT = [None] * NCH
            def dma_x(c):
                xT[c] = xpool.tile([128, KT, CH], BF16, tag="xT", name="xT")
                nc.sync.dma_start(xT[c][:], xr[:, c])
            dma_x(0)
            wq_sb = wpool.tile([128, CT, KT, 128], BF16, tag="wq")
            wk_sb = wpool.tile([128, CT, KC, 128], BF16, tag="wk")
            nc.sync.dma_start(wq_sb[:, 0:1], wqr[:, 0:1])
            yt_sb = consts.tile([128, KC, SKV], BF16, tag="yt")
            nc.sync.dma_start(yt_sb[:], yp.ap().rearrange("p (kt s) -> p kt s", kt=KC))
            nc.sync.dma_start(wk_sb[:, 0:2], wkr[:, 0:2])
            bq32_sb = consts.tile([128, CT], F32, tag="bq32")
            nc.sync.dma_start(bq32_sb[:], bq32_d.ap())
            nc.sync.dma_start(wq_sb[:, 1:2], wqr[:, 1:2])
            bk8_sb = consts.tile([128, CT], F32, tag="bk8")
            nc.sync.dma_start(bk8_sb[:], bk8_d.ap())
            nc.sync.dma_start(wq_sb[:, 2:4], wqr[:, 2:4])
            nc.sync.dma_start(wk_sb[:, 2:4], wkr[:, 2:4])
            nc.sync.dma_start(wq_sb[:, 4:8], wqr[:, 4:8])
            nc.sync.dma_start(wk_sb[:, 4:8], wkr[:, 4:8])
            dma_x(1)
            wv_sb = wpool.tile([128, KC, D], BF16, tag="wv")
            nc.sync.dma_start(wv_sb[:], wv_d.ap().rearrange("p (kt n) -> p kt n", kt=KC))
            bv_sb = consts.tile([1, D], BF16, tag="bv")
            nc.sync.dma_start(bv_sb[:], bv_d.ap())
            wo_sb = wpool.tile([128, KT, D], BF16, tag="wo")
            nc.sync.dma_start(wo_sb[:], wo_d.ap().rearrange("p (kt n) -> p kt n", kt=KT))
            boT_sb = consts.tile([128, CT], F32, tag="boT")
            nc.sync.dma_start(boT_sb[:], boT_d.ap())

            kT_sb = consts.tile([128, CT, SKV], BF16, tag="kT")
            v_aug = consts.tile([SKV, H, DH], BF16, tag="v")

            # PE warm-up: ~3us of throwaway matmuls with no DMA dependency,
            # so the HAM clock gate is released before the first real matmul
            # and the PE isn't idle while the first inputs stream in
            psw = pq.tile([128, CH], F32, tag="mm", name="warm")
            for _ in range(12):
                nc.tensor.matmul(psw[0:64, :], ones64[:], scr[:],
                                 start=True, stop=True)

            # k projection: kT[c, s] = sum_k wk[k, c] yT[k, s]; fold (.+bk)/8
            def k_proj(ct0, ct1):
                for ct in range(ct0, ct1):
                    psk = pq.tile([128, CH], F32, tag="mm", name="psk")
                    for kt in range(KC):
                        nc.tensor.matmul(
                            psk[:, 0:SKV],
                            wk_sb[:, ct, kt],
                            yt_sb[:, kt, :],
                            start=(kt == 0),
                            stop=(kt == KC - 1),
                        )
                    nc.scalar.activation(
                        kT_sb[:, ct, :],
                        psk[:, 0:SKV],
                        AF.Identity,
                        scale=0.125,
                        bias=bk8_sb[:, ct:ct + 1],
                    )

            def v_proj():
                for n in range(2):
                    psv = pq.tile([128, CH], F32, tag="mm", name="psv")
                    for kt in range(KC):
                        nc.tensor.matmul(
                            psv[0:SKV, :],
                            yt_sb[:, kt, :],
                            wv_sb[:, kt, n * 512:(n + 1) * 512],
                            start=(kt == 0),
                            stop=False,
                        )
                    nc.tensor.matmul(
                        psv[0:SKV, :],
                        ones77r[:],
                        bv_sb[0:1, n * 512:(n + 1) * 512],
                        start=False,
                        stop=True,
                    )
                    for jj in range(8):
                        h = n * 8 + jj
                        nc.any.tensor_copy(v_aug[:, h, :], psv[0:SKV, jj * DH:(jj + 1) * DH])

            # ---- software-pipelined main loop ----
            qT = [None] * NCH
            e_ch = [None] * NCH
            rb_ch = [[None] * HP for _ in range(NCH)]

            def sums_pair(c, hp):
                # exp-sums for head pair hp of chunk c: all-ones [77,64]
                # stationary, the two heads col-tiled into rows 0:64/64:128
                # of one PSUM bank (concurrent in the PE array)
                sp = att.tile([128, CH], F32, tag="att", name="sums")
                for h2 in range(2):
                    h = 2 * hp + h2
                    nc.tensor.matmul(
                        sp[h2 * 64:(h2 + 1) * 64, :],
                        ones64[:],
                        e_ch[c][:, h, :],
                        start=True, stop=True,
                    )
                rb_ch[c][hp] = rbpool.tile([128, CH], F32, tag="rb", name="rb")
                nc.vector.reciprocal_approx_fast(rb_ch[c][hp][:], sp[:])

            for i in range(NCH + 1):
                j = i - 1  # pass-B chunk

                if i + 2 < NCH:
                    dma_x(i + 2)

                if i < NCH:
                    # pass A: q^T projection; paired scores+exp trail one granule
                    qT[i] = qpool.tile([128, CT, CH], BF16, tag="qT", name="qT")
                    e_ch[i] = epool.tile([SKV, H, CH], BF16, tag="e", name="e_ch")

                    def scores_pair(g):
                        pssc = psc.tile([SKV, 2, CH], F32, tag="sc", name="pssc")
                        for half in range(2):
                            h = 2 * g + half
                            nc.tensor.matmul(
                                pssc[:, half, :],
                                kT_sb[(h % 2) * 64:(h % 2) * 64 + 64, h // 2, :],
                                qT[i][(h % 2) * 64:(h % 2) * 64 + 64, h // 2, :],
                                start=True, stop=True,
                            )
                        nc.scalar.activation(
                            e_ch[i][:, 2 * g:2 * g + 2, :], pssc[:], AF.Exp,
                        )

                    lag = 2 if i == 0 else 1  # iter 0: k-proj interleaves, scores trail it
                    for g in range(CT):
                        psq = pq.tile([128, CH], F32, tag="mm", name="psq")
                        for kt in range(KT):
                            nc.tensor.matmul(
                                psq[:],
                                wq_sb[:, g, kt],
                                xT[i][:, kt, :],
                                start=(kt == 0),
                                stop=(kt == KT - 1),
                            )
                        nc.vector.tensor_scalar_add(
                            qT[i][:, g, :], psq[:], bq32_sb[:, g:g + 1],
                        )
                        if i == 0 and 2 <= g < 6:
                            k_proj(2 * (g - 2), 2 * (g - 2) + 2)
                        if g >= lag:
                            scores_pair(g - lag)
                    for gg in range(CT - lag, CT):
                        scores_pair(gg)
                    if i == 1:
                        v_proj()

                if j >= 0:
                    # pass B: PV pairs + normalization muls (rb from iter j)
                    aoT = aopool.tile([128, KT, CH], BF16, tag="aoT", name="aoT")
                    for hp in range(HP):
                        pspv = att.tile([128, CH], F32, tag="att", name="pspv")
                        for h2 in range(2):
                            h = 2 * hp + h2
                            nc.tensor.matmul(
                                pspv[h2 * 64:(h2 + 1) * 64, :],
                                v_aug[:, h, :],
                                e_ch[j][:, h, :],
                                start=True, stop=True,
                            )
                        nc.vector.tensor_mul(aoT[:, hp, :], pspv[:], rb_ch[j][hp][:])

                    # transposed O-projection: outT[n,q] = wo_n^T @ aoT (+bo
                    # via ACT bias); chunk i's exp-sum pairs interleave
                    # between O-groups
                    for n in range(CT):
                        psoT = pq.tile([128, CH], F32, tag="mm", name="psoT")
                        for kt in range(KT):
                            nc.tensor.matmul(
                                psoT[:],
                                wo_sb[:, kt, n * 128:(n + 1) * 128],
                                aoT[:, kt, :],
                                start=(kt == 0),
                                stop=(kt == KT - 1),
                            )
                        o_sb = opool.tile([128, CH], F32, tag="o")
                        nc.vector.tensor_scalar_add(
                            o_sb[:], psoT[:], boT_sb[:, n:n + 1],
                        )
                        # out stores ride the Activation engine's HW DGE
                        # queue, leaving the sync queue to input prefetch
                        if j == NCH - 1:
                            # split the trailing stores across BOTH queues so
                            # the kernel's tail isn't one serial transfer deep
                            nc.scalar.dma_start(
                                out_d.ap()[n * 128:n * 128 + 64,
                                           j * CH:(j + 1) * CH],
                                o_sb[0:64, :],
                            )
                            nc.sync.dma_start(
                                out_d.ap()[n * 128 + 64:(n + 1) * 128,
                                           j * CH:(j + 1) * CH],
                                o_sb[64:128, :],
                            )
                        else:
                            nc.scalar.dma_start(
                                out_d.ap()[n * 128:(n + 1) * 128,
                                           j * CH:(j + 1) * CH],
                                o_sb[:],
                            )
                        if i < NCH:
                            sums_pair(i, n)
                elif i == 0:
                    pass
                if i == 0:
                    for hp in range(HP):
                        sums_pair(0, hp)

    nc.compile()
    return nc


def _get_nc():
    if "nc" not in _CACHED:
        _CACHED["nc"] = _build()
    return _CACHED["nc"]


def _prep_in_maps(x, y, wq, bq, wk, bk, wv, bv, wo, bo):
    x = np.asarray(x)
    y = np.asarray(y)
    bf = ml_dtypes.bfloat16

    def pack_rows(m, ktn):
        return np.ascontiguousarray(
            m.reshape(ktn, 128, m.shape[1]).transpose(1, 0, 2)
        ).astype(bf)

    wq_a, wk_a, wv_a, wo_a = (np.asarray(w) for w in (wq, wk, wv, wo))
    wq_p = pack_rows(wq_a, KT).reshape(128, KT, CT, 128).transpose(0, 2, 1, 3)
    wq_p = np.ascontiguousarray(wq_p).reshape(128, CT * KT * 128)
    wk_p = pack_rows(wk_a, KC).reshape(128, KC, CT, 128).transpose(0, 2, 1, 3)
    wk_p = np.ascontiguousarray(wk_p).reshape(128, CT * KC * 128)
    wv_p = pack_rows(wv_a, KC).reshape(128, KC * D)
    wo_p = pack_rows(wo_a, KT).reshape(128, KT * D)

    bq32 = np.ascontiguousarray(
        np.asarray(bq).astype(np.float32).reshape(CT, 128).T)
    bk8 = np.ascontiguousarray(
        (np.asarray(bk).astype(np.float32) * 0.125).reshape(CT, 128).T)
    boT = np.ascontiguousarray(
        np.asarray(bo).astype(np.float32).reshape(CT, 128).T)
    bv_b = np.asarray(bv).reshape(1, D).astype(bf)

    in_maps = []
    for b in range(B):
        xt = x[b].T  # (D, SQ)
        xpk = xt.reshape(KT, 128, NCH, CH).transpose(1, 2, 0, 3)
        xpk = np.ascontiguousarray(xpk).astype(bf).reshape(128, NCH * KT * CH)
        yt = y[b].T  # (DC, SKV)
        ypk = yt.reshape(KC, 128, SKV).transpose(1, 0, 2)
        ypk = np.ascontiguousarray(ypk).astype(bf).reshape(128, KC * SKV)
        in_maps.append({
            "xp": xpk, "yp": ypk,
            "wq": wq_p, "wk": wk_p, "wv": wv_p, "wo": wo_p,
            "bq32": bq32, "bk8": bk8, "bv": bv_b, "boT": boT,
        })
    return in_maps


def kernel(x, y, wq, bq, wk, bk, wv, bv, wo, bo):
    in_maps = _prep_in_maps(x, y, wq, bq, wk, bk, wv, bv, wo, bo)
    nc = _get_nc()
    res = bass_utils.run_bass_kernel_spmd(nc, in_maps, core_ids=list(range(B)))
    out = np.stack(
        [np.ascontiguousarray(res.results[b]["out"].T) for b in range(B)], axis=0
    )
    return out.astype(np.float32)


# revision 24
# speedup vs baseline: 1.2004x; 1.2004x over previous
"""Cross-attention kernel for TRN2, SPMD over 8 NeuronCores.

Problem: B=8, SQ=4096, SKV=77, D_EMBED=1024, D_CROSS=768, H=16, DH=64.
  q = x @ wq + bq ; k = y @ wk + bk ; v = y @ wv + bv
  out = softmax(q k^T / 8) v @ wo + bo

Sharding: pure data-parallel over batch (1 batch element per core, no
collectives). Host pre-packs every DRAM operand into the exact
[partition, tile...] layout the device consumes (multi-KB contiguous DMA
lines), and the O-projection emits the output TRANSPOSED (outT[d, q]) so
its PSUM drain is an ACT activation with per-partition bias; the host
transposes back.

Compute dtype: bf16 operands (host-cast), fp32 PSUM accumulation, fp32 out.

Schedule: two-stage software pipeline over 512-wide query chunks.
Iteration i runs pass A for chunk i (Q-projection with paired score
matmuls + exp trailing one granule behind) then pass B for chunk i-1
(PV pairs + normalization muls, O-projection with chunk i's exp-sum
pairs interleaved between O-groups so their reciprocals have a whole
O-phase of DVE slack).

Softmax without max-subtraction (scores are O(5) here; exp stays inside
fp32/bf16 range):
  scoresT[s,q] = k'_h @ q_h^T with k' = (k + bk)/8 folded at k-projection
  e = exp(scoresT)  (bf16)
  rb[d,q] = 1 / (ones^T @ e)  per-head exp-sums broadcast over the head's
            64-row block by an all-ones [77,64] stationary, col-tiled so
            a head PAIR costs one 512-column stream
  aoT[d,q] = (v_h^T @ e) * rb   (normalization commutes with PV)
  outT[n,q] = wo_slice^T @ aoT  (+bo via ACT bias at the PSUM drain)

Engine budget per iteration (steady state): PE ~32.5us (the bottleneck:
projections 27.3, scores/sums/PV 1.7 each), ACT ~16us (exps + qT/outT
drains), DVE ~10us (reciprocals + aoT muls), PSUM 8 banks = pq(4) +
scores(1x2) + sums/PV shared(2x1).
"""

import numpy as np
import ml_dtypes

import concourse.bass as bass
import concourse.mybir as mybir
import concourse.tile as tile
from concourse import bacc
from concourse import bass_utils

F32 = mybir.dt.float32
BF16 = mybir.dt.bfloat16
AF = mybir.ActivationFunctionType

B = 8
SQ = 4096
SKV = 77
D = 1024
DC = 768
H = 16
DH = 64
KT = D // 128    # 8 embed k-tiles
KC = DC // 128   # 6 cross k-tiles
CT = D // 128    # 8 column tiles of the 1024-wide projections
CH = 512         # query chunk
NCH = SQ // CH   # 8 chunks
HP = H // 2      # 8 head pairs

_CACHED = {}


def _build():
    nc = bacc.Bacc("TRN2", target_bir_lowering=False, debug=False, num_devices=B)

    xp = nc.dram_tensor("xp", (128, NCH * KT * CH), BF16, kind="ExternalInput")
    yp = nc.dram_tensor("yp", (128, KC * SKV), BF16, kind="ExternalInput")
    wq_d = nc.dram_tensor("wq", (128, CT * KT * 128), BF16, kind="ExternalInput")
    wk_d = nc.dram_tensor("wk", (128, CT * KC * 128), BF16, kind="ExternalInput")
    wv_d = nc.dram_tensor("wv", (128, KC * D), BF16, kind="ExternalInput")
    wo_d = nc.dram_tensor("wo", (128, KT * D), BF16, kind="ExternalInput")
    bq32_d = nc.dram_tensor("bq32", (128, CT), F32, kind="ExternalInput")
    bk8_d = nc.dram_tensor("bk8", (128, CT), F32, kind="ExternalInput")
    bv_d = nc.dram_tensor("bv", (1, D), BF16, kind="ExternalInput")
    boT_d = nc.dram_tensor("boT", (128, CT), F32, kind="ExternalInput")
    out_d = nc.dram_tensor("out", (D, SQ), F32, kind="ExternalOutput")

    with tile.TileContext(nc) as tc:
        with (
            tc.tile_pool(name="consts", bufs=1) as consts,
            tc.tile_pool(name="wpool", bufs=1) as wpool,
            tc.tile_pool(name="xpool", bufs=2) as xpool,
            tc.tile_pool(name="qpool", bufs=2) as qpool,
            tc.tile_pool(name="epool", bufs=2) as epool,
            tc.tile_pool(name="rbpool", bufs=16) as rbpool,
            tc.tile_pool(name="aopool", bufs=2) as aopool,
            tc.tile_pool(name="opool", bufs=3) as opool,
            tc.tile_pool(name="pq", bufs=3, space="PSUM") as pq,
            tc.tile_pool(name="psc", bufs=1, space="PSUM") as psc,
            tc.tile_pool(name="att", bufs=3, space="PSUM") as att,
        ):
            xr = xp.ap().rearrange("p (c kt q) -> p c kt q", c=NCH, kt=KT)
            wqr = wq_d.ap().rearrange("p (g kt n) -> p g kt n", g=CT, kt=KT)
            wkr = wk_d.ap().rearrange("p (g kt n) -> p g kt n", g=CT, kt=KC)

            # device-generated constants (no DMA dependency)
            ones77r = consts.tile([1, SKV], BF16, tag="ones77r")
            nc.vector.memset(ones77r[:], 1.0)
            ones64 = consts.tile([SKV, 64], BF16, tag="ones64")
            nc.vector.memset(ones64[:], 1.0)
            scr = consts.tile([SKV, CH], BF16, tag="scr")
            nc.vector.memset(scr[:], 0.0)

            # ---- startup DMAs: Qproj(0) inputs first (the single sync HW
            # queue transfers in issue order at ~320 GB/s, so order = priority)
            xT = [None] * NCH
            def dma_x(c):
                xT[c] = xpool.tile([128, KT, CH], BF16, tag="xT", name="xT")
                nc.sync.dma_start(xT[c][:], xr[:, c])
            dma_x(0)
            wq_sb = wpool.tile([128, CT, KT, 128], BF16, tag="wq")
            wk_sb = wpool.tile([128, CT, KC, 128], BF16, tag="wk")
            nc.sync.dma_start(wq_sb[:, 0:1], wqr[:, 0:1])
            yt_sb = consts.tile([128, KC, SKV], BF16, tag="yt")
            nc.sync.dma_start(yt_sb[:], yp.ap().rearrange("p (kt s) -> p kt s", kt=KC))
            nc.sync.dma_start(wk_sb[:, 0:2], wkr[:, 0:2])
            bq32_sb = consts.tile([128, CT], F32, tag="bq32")
            nc.sync.dma_start(bq32_sb[:], bq32_d.ap())
            nc.sync.dma_start(wq_sb[:, 1:2], wqr[:, 1:2])
            bk8_sb = consts.tile([128, CT], F32, tag="bk8")
            nc.sync.dma_start(bk8_sb[:], bk8_d.ap())
            nc.sync.dma_start(wq_sb[:, 2:4], wqr[:, 2:4])
            nc.sync.dma_start(wk_sb[:, 2:4], wkr[:, 2:4])
            nc.sync.dma_start(wq_sb[:, 4:8], wqr[:, 4:8])
            nc.sync.dma_start(wk_sb[:, 4:8], wkr[:, 4:8])
            dma_x(1)
            wv_sb = wpool.tile([128, KC, D], BF16, tag="wv")
            nc.sync.dma_start(wv_sb[:], wv_d.ap().rearrange("p (kt n) -> p kt n", kt=KC))
            bv_sb = consts.tile([1, D], BF16, tag="bv")
            nc.sync.dma_start(bv_sb[:], bv_d.ap())
            wo_sb = wpool.tile([128, KT, D], BF16, tag="wo")
            nc.sync.dma_start(wo_sb[:], wo_d.ap().rearrange("p (kt n) -> p kt n", kt=KT))
            boT_sb = consts.tile([128, CT], F32, tag="boT")
            nc.sync.dma_start(boT_sb[:], boT_d.ap())

            kT_sb = consts.tile([128, CT, SKV], BF16, tag="kT")
            v_aug = consts.tile([SKV, H, DH], BF16, tag="v")

            # PE warm-up: ~3us of throwaway matmuls with no DMA dependency,
            # so the HAM clock gate is released before the first real matmul
            # and the PE isn't idle while the first inputs stream in
            psw = pq.tile([128, CH], F32, tag="mm", name="warm")
            for _ in range(12):
                nc.tensor.matmul(psw[0:64, :], ones64[:], scr[:],
                                 start=True, stop=True)

            # k projection: kT[c, s] = sum_k wk[k, c] yT[k, s]; fold (.+bk)/8
            def k_proj(ct0, ct1):
                for ct in range(ct0, ct1):
                    psk = pq.tile([128, CH], F32, tag="mm", name="psk")
                    for kt in range(KC):
                        nc.tensor.matmul(
                            psk[:, 0:SKV],
                            wk_sb[:, ct, kt],
                            yt_sb[:, kt, :],
                            start=(kt == 0),
                            stop=(kt == KC - 1),
                        )
                    nc.scalar.activation(
                        kT_sb[:, ct, :],
                        psk[:, 0:SKV],
                        AF.Identity,
                        scale=0.125,
                        bias=bk8_sb[:, ct:ct + 1],
                    )

            def v_proj():
                for n in range(2):
                    psv = pq.tile([128, CH], F32, tag="mm", name="psv")
                    for kt in range(KC):
                        nc.tensor.matmul(
                            psv[0:SKV, :],
                            yt_sb[:, kt, :],
                            wv_sb[:, kt, n * 512:(n + 1) * 512],
                            start=(kt == 0),
                            stop=False,
                        )
                    nc.tensor.matmul(
                        psv[0:SKV, :],
                        ones77r[:],
                        bv_sb[0:1, n * 512:(n + 1) * 512],
                        start=False,
                        stop=True,
                    )
                    for jj in range(8):
                        h = n * 8 + jj
                        nc.any.tensor_copy(v_aug[:, h, :], psv[0:SKV, jj * DH:(jj + 1) * DH])

            # ---- software-pipelined main loop ----
            qT = [None] * NCH
            e_ch = [None] * NCH
            rb_ch = [[None] * HP for _ in range(NCH)]

            def sums_pair(c, hp):
                # exp-sums for head pair hp of chunk c: all-ones [77,64]
                # stationary, the two heads col-tiled into rows 0:64/64:128
                # of one PSUM bank (concurrent in the PE array)
                sp = att.tile([128, CH], F32, tag="att", name="sums")
                for h2 in range(2):
                    h = 2 * hp + h2
                    nc.tensor.matmul(
                        sp[h2 * 64:(h2 + 1) * 64, :],
                        ones64[:],
                        e_ch[c][:, h, :],
                        start=True, stop=True,
                    )
                rb_ch[c][hp] = rbpool.tile([128, CH], F32, tag="rb", name="rb")
                nc.vector.reciprocal_approx_fast(rb_ch[c][hp][:], sp[:])

            for i in range(NCH + 1):
                j = i - 1  # pass-B chunk

                if i + 2 < NCH:
                    dma_x(i + 2)

                if i < NCH:
                    # pass A: q^T projection; paired scores+exp trail one granule
                    qT[i] = qpool.tile([128, CT, CH], BF16, tag="qT", name="qT")
                    e_ch[i] = epool.tile([SKV, H, CH], BF16, tag="e", name="e_ch")

                    def scores_pair(g):
                        pssc = psc.tile([SKV, 2, CH], F32, tag="sc", name="pssc")
                        for half in range(2):
                            h = 2 * g + half
                            nc.tensor.matmul(
                                pssc[:, half, :],
                                kT_sb[(h % 2) * 64:(h % 2) * 64 + 64, h // 2, :],
                                qT[i][(h % 2) * 64:(h % 2) * 64 + 64, h // 2, :],
                                start=True, stop=True,
                            )
                        nc.scalar.activation(
                            e_ch[i][:, 2 * g:2 * g + 2, :], pssc[:], AF.Exp,
                        )

                    lag = 2 if i == 0 else 1  # iter 0: k-proj interleaves, scores trail it
                    for g in range(CT):
                        psq = pq.tile([128, CH], F32, tag="mm", name="psq")
                        for kt in range(KT):
                            nc.tensor.matmul(
                                psq[:],
                                wq_sb[:, g, kt],
                                xT[i][:, kt, :],
                                start=(kt == 0),
                                stop=(kt == KT - 1),
                            )
                        nc.vector.tensor_scalar_add(
                            qT[i][:, g, :], psq[:], bq32_sb[:, g:g + 1],
                        )
                        if i == 0 and 2 <= g < 6:
                            k_proj(2 * (g - 2), 2 * (g - 2) + 2)
                        if g >= lag:
                            scores_pair(g - lag)
                    for gg in range(CT - lag, CT):
                        scores_pair(gg)
                    if i == 1:
                        v_proj()

                if j >= 0:
                    # pass B: PV pairs + normalization muls (rb from iter j)
                    aoT = aopool.tile([128, KT, CH], BF16, tag="aoT", name="aoT")
                    for hp in range(HP):
                        pspv = att.tile([128, CH], F32, tag="att", name="pspv")
                        for h2 in range(2):
                            h = 2 * hp + h2
                            nc.tensor.matmul(
                                pspv[h2 * 64:(h2 + 1) * 64, :],
                                v_aug[:, h, :],
                                e_ch[j][:, h, :],
                                start=True, stop=True,
                            )
                        nc.vector.tensor_mul(aoT[:, hp, :], pspv[:], rb_ch[j][hp][:])

                    # transposed O-projection: outT[n,q] = wo_n^T @ aoT (+bo
                    # via ACT bias); chunk i's exp-sum pairs interleave
                    # between O-groups
                    for n in range(CT):
                        psoT = pq.tile([128, CH], F32, tag="mm", name="psoT")
                        for kt in range(KT):
                            nc.tensor.matmul(
                                psoT[:],
                                wo_sb[:, kt, n * 128:(n + 1) * 128],
                                aoT[:, kt, :],
                                start=(kt == 0),
                                stop=(kt == KT - 1),
                            )
                        o_sb = opool.tile([128, CH], F32, tag="o")
                        nc.vector.tensor_scalar_add(
                            o_sb[:], psoT[:], boT_sb[:, n:n + 1],
                        )
                        if j == NCH - 1:
                            # split the trailing stores so the kernel's tail
                            # isn't one serial 256KB transfer deep
                            nc.sync.dma_start(
                                out_d.ap()[n * 128:n * 128 + 64,
                                           j * CH:(j + 1) * CH],
                                o_sb[0:64, :],
                            )
                            nc.sync.dma_start(
                                out_d.ap()[n * 128 + 64:(n + 1) * 128,
                                           j * CH:(j + 1) * CH],
                                o_sb[64:128, :],
                            )
                        else:
                            nc.sync.dma_start(
                                out_d.ap()[n * 128:(n + 1) * 128,
                                           j * CH:(j + 1) * CH],
                                o_sb[:],
                            )
                        if i < NCH:
                            sums_pair(i, n)
                elif i == 0:
                    pass
                if i == 0:
                    for hp in range(HP):
                        sums_pair(0, hp)

    nc.compile()
    return nc


def _get_nc():
    if "nc" not in _CACHED:
        _CACHED["nc"] = _build()
    return _CACHED["nc"]


def _prep_in_maps(x, y, wq, bq, wk, bk, wv, bv, wo, bo):
    x = np.asarray(x)
    y = np.asarray(y)
    bf = ml_dtypes.bfloat16

    def pack_rows(m, ktn):
        return np.ascontiguousarray(
            m.reshape(ktn, 128, m.shape[1]).transpose(1, 0, 2)
        ).astype(bf)

    wq_a, wk_a, wv_a, wo_a = (np.asarray(w) for w in (wq, wk, wv, wo))
    wq_p = pack_rows(wq_a, KT).reshape(128, KT, CT, 128).transpose(0, 2, 1, 3)
    wq_p = np.ascontiguousarray(wq_p).reshape(128, CT * KT * 128)
    wk_p = pack_rows(wk_a, KC).reshape(128, KC, CT, 128).transpose(0, 2, 1, 3)
    wk_p = np.ascontiguousarray(wk_p).reshape(128, CT * KC * 128)
    wv_p = pack_rows(wv_a, KC).reshape(128, KC * D)
    wo_p = pack_rows(wo_a, KT).reshape(128, KT * D)

    bq32 = np.ascontiguousarray(
        np.asarray(bq).astype(np.float32).reshape(CT, 128).T)
    bk8 = np.ascontiguousarray(
        (np.asarray(bk).astype(np.float32) * 0.125).reshape(CT, 128).T)
    boT = np.ascontiguousarray(
        np.asarray(bo).astype(np.float32).reshape(CT, 128).T)
    bv_b = np.asarray(bv).reshape(1, D).astype(bf)

    in_maps = []
    for b in range(B):
        xt = x[b].T  # (D, SQ)
        xpk = xt.reshape(KT, 128, NCH, CH).transpose(1, 2, 0, 3)
        xpk = np.ascontiguousarray(xpk).astype(bf).reshape(128, NCH * KT * CH)
        yt = y[b].T  # (DC, SKV)
        ypk = yt.reshape(KC, 128, SKV).transpose(1, 0, 2)
        ypk = np.ascontiguousarray(ypk).astype(bf).reshape(128, KC * SKV)
        in_maps.append({
            "xp": xpk, "yp": ypk,
            "wq": wq_p, "wk": wk_p, "wv": wv_p, "wo": wo_p,
            "bq32": bq32, "bk8": bk8, "bv": bv_b, "boT": boT,
        })
    return in_maps


def kernel(x, y, wq, bq, wk, bk, wv, bv, wo, bo):
    in_maps = _prep_in_maps(x, y, wq, bq, wk, bk, wv, bv, wo, bo)
    nc = _get_nc()
    res = bass_utils.run_bass_kernel_spmd(nc, in_maps, core_ids=list(range(B)))
    out = np.stack(
        [np.ascontiguousarray(res.results[b]["out"].T) for b in range(B)], axis=0
    )
    return out.astype(np.float32)
